# revision 24
# baseline (speedup 1.0000x reference)
"""Trainium2 Bass kernel for BasicEuclideanDistModel (gnn_message_passing).

Math:
  result = sum_e (beta - ||dz_e + dv_e t_e||)
           - dt * sum_{i<j, s} exp(beta - ||z_i(t_s) - z_j(t_s)||)

Device strategy (8 cores, one uniform SPMD program; per-core variation
lives entirely in the input DATA):

* Non-event term, upper-triangular only (~2x less work than full NxN):
  d^2(i,j,s) = F_i(s) . G_j (K=8 fp32r inner product, see below). The
  triangle is cut into 40 column-chunks of 512 (16 "diag" chunks that
  start at a tile's diagonal block + 24 continuations, sentinel-padded
  to 512). Every core gets exactly 2 diag + 3 continuation chunks ->
  [128, 2560] per sample; the host packs per-core i-slot rows (zv_i)
  and j-column node lists (zv_j). Pad columns hold a far-away sentinel
  node so exp(-d) is exactly 0. Diag chunks sit at w[:, 0:256]; their
  exp uses bias=-ln2 (halves the double-counted in-block pairs; the
  128 self-pairs per block contribute 0.5 each, host subtracts).
    F_i(s) = [r_i(s), 1, t_s, t_s^2, -2x_i(s), -2t_s x_i(s), -2y_i(s), -2t_s y_i(s)]
    G_j    = [1,  a_j, b_j, c_j,  zx_j, vx_j, zy_j, vy_j]
  ACT sqrt then exp with fused per-partition row sums (one accum col
  per (s, diag/strip)).

* Event term: events globally sorted by u; each partition owns 196
  consecutive events, so its u values span < 16 consecutive nodes.
  One windowed-row table zw[n] = zv[n:n+16].T (c-major, 256B rows)
  serves both sides:
    - u side: ONE 256B gather per partition (128 descriptors) of the
      window starting at that partition's first u; a [16]-one-hot
      (host input) selects each event's u row on DVE (bf16).
    - v side: one 256B gather per event slot (25088 descriptors, 4
      ops across the 4 SWDGE queues); only the first c-major column
      (the row's own node) is read back.
  DVE distance algebra in f32, ACT sqrt with fused row-sum.

* beta folded in on host: sum exp(beta-d) = e^beta sum exp(-d);
  sum(beta-d) = E beta - sum d. Host combines the 8 cores' [128, 24]
  partial-sum tensors (pure unshard/reduction).
"""

import math
import os

import numpy as np


def _import_concourse():
    try:
        import concourse  # noqa: F401
    except ImportError:
        import sys

        for p in ("/opt/trn_rl_repo", "/root/.axon_site/_ro/trn_rl_repo"):
            if os.path.isdir(p) and p not in sys.path:
                sys.path.insert(0, p)


_import_concourse()

from contextlib import ExitStack  # noqa: E402

import concourse.bacc as bacc  # noqa: E402
import concourse.bass as bass  # noqa: E402
import concourse.mybir as mybir  # noqa: E402
import concourse.tile as tile  # noqa: E402
from concourse.tile_rust import add_dep_helper  # noqa: E402

N = 2048          # nodes
S = 10            # Riemann samples
NCORES = 8
NSLOT = 5         # 512-col j-chunks per core (2 diag + 3 continuation)
JCOLS = NSLOT * 512                  # 2560 j columns per sample
JT = JCOLS // 128                    # 20 column-tiles for G features
EV_PER_CORE = 200000 // NCORES       # 25000 real events per core
C_EV = 196        # event slots per partition (128*196 = 25088 >= 25000)
EV_CHUNKS = 4     # v-side gather ops per core (one per SWDGE queue)
EV_CC = C_EV // EV_CHUNKS            # 49 event columns per chunk
EV_PER_CHUNK = 128 * EV_CC           # 6272
WIN = 4           # u-window nodes (max observed span is 3)
GELEM = 64        # gather element size in f32 (256B rows; first 4*WIN used)
LN2 = math.log(2.0)

F32 = mybir.dt.float32
F32R = mybir.dt.float32r
BF16 = mybir.dt.bfloat16
I16 = mybir.dt.int16
AF = mybir.ActivationFunctionType
OP = mybir.AluOpType

_CACHE: dict = {}


def _tt(nc, out, in0, in1, op):
    return nc.vector.tensor_tensor(out, in0, in1, op=op)


# ---- static chunk assignment (core c -> 5 chunks of the triangle) ----
def _chunk_plan():
    """Returns per-core [(tile, col_start, ncols_real, is_first) x 5].
    Chunk k of tile t covers j-columns [t*128 + k*512, ...) of the strip
    j in [t*128, 2048). Slot 0/1 are diag chunks (start at the tile's
    own block), slots 2-4 are continuations."""
    firsts = []
    conts = []
    for t in range(16):
        w = (16 - t) * 128
        nch = (w + 511) // 512
        for k in range(nch):
            start = t * 128 + k * 512
            ncols = min(512, w - k * 512)
            (firsts if k == 0 else conts).append((t, start, ncols, k == 0))
    assert len(firsts) == 16 and len(conts) == 24
    plan = []
    for c in range(NCORES):
        plan.append([firsts[c], firsts[15 - c]] + conts[3 * c : 3 * c + 3])
    return plan


_PLAN = _chunk_plan()


def _build():
    if "nc" in _CACHE:
        return _CACHE["nc"]

    nc = bacc.Bacc(
        "TRN2", target_bir_lowering=False, debug=False, enable_asserts=False,
        num_swdge_queues=4,
    )

    zw_d = nc.dram_tensor("zw", [N, GELEM], F32, kind="ExternalInput").ap()
    zvj_d = nc.dram_tensor("zv_j", [JCOLS, 4], F32, kind="ExternalInput").ap()
    zvi_d = nc.dram_tensor("zv_i", [NSLOT * 128, 4], F32, kind="ExternalInput").ap()
    # int16 indices, dma_gather wrap: op ch's index k lives at
    # [k % 16, ch, k // 16], replicated down all 8 blocks of 16 partitions
    ev_u = nc.dram_tensor("ev_u", [128, 1, 128 // 16], I16, kind="ExternalInput").ap()
    ev_v = nc.dram_tensor(
        "ev_v", [128, EV_CHUNKS, EV_PER_CHUNK // 16], I16, kind="ExternalInput"
    ).ap()
    ev_oh = nc.dram_tensor("ev_oh", [128, C_EV, WIN], BF16, kind="ExternalInput").ap()
    ev_t = nc.dram_tensor("ev_t", [128, C_EV], F32, kind="ExternalInput").ap()
    tb_d = nc.dram_tensor("tb", [128, S], F32, kind="ExternalInput").ap()
    t2b_d = nc.dram_tensor("t2b", [128, S], F32, kind="ExternalInput").ap()
    ident_d = nc.dram_tensor("ident", [128, 128], F32, kind="ExternalInput").ap()
    out_p = nc.dram_tensor("out_p", [128, 24], F32, kind="ExternalOutput").ap()

    with tile.TileContext(nc) as tc, ExitStack() as ctx:
        cpool = ctx.enter_context(tc.tile_pool(name="const", bufs=1))
        evpool = ctx.enter_context(tc.tile_pool(name="ev", bufs=1))

        # ---------------- input loads ----------------
        # pairwise-critical loads go FIRST on the Sync (HWDGE) queue so their
        # completion sems are never sequenced after the (slow) gather sems;
        # the event index loads ride the Pool engine itself (SWDGE), putting
        # them in-stream with the gathers that consume them
        zvj_sb = cpool.tile([128, JT, 4], F32)       # this core's j columns
        nc.sync.dma_start(zvj_sb[:], zvj_d.rearrange("(c p) d -> p c d", p=128))
        zvi_sb = cpool.tile([128, NSLOT, 4], F32)    # this core's i slots
        nc.sync.dma_start(zvi_sb[:], zvi_d.rearrange("(c p) d -> p c d", p=128))
        tb = cpool.tile([128, S], F32)
        nc.sync.dma_start(tb[:], tb_d)
        t2b = cpool.tile([128, S], F32)
        nc.sync.dma_start(t2b[:], t2b_d)
        # identity comes from the host: building it with make_identity would
        # occupy the gpsimd engine ahead of the gather descriptor generation
        ident = cpool.tile([128, 128], F32)
        nc.sync.dma_start(ident[:], ident_d)
        u_sb = evpool.tile([128, 1, 128 // 16], I16)
        nc.gpsimd.dma_start(u_sb[:], ev_u)
        v_sb = evpool.tile([128, EV_CHUNKS, EV_PER_CHUNK // 16], I16)
        nc.gpsimd.dma_start(v_sb[:], ev_v)
        oh_sb = evpool.tile([128, C_EV, WIN], BF16)
        nc.gpsimd.dma_start(oh_sb[:], ev_oh)
        t_sb = evpool.tile([128, C_EV], F32)
        nc.gpsimd.dma_start(t_sb[:], ev_t)

        acc = cpool.tile([128, 24], F32)
        nc.vector.memset(acc[:], 0.0)

        # ---------------- event gathers (all Pool-engine work upfront) ----
        # u side: one 256B window row per partition (nodes [u0, u0+16),
        # c-major [4, 16]); v side: one row per event slot, of which only
        # column 0 of the c-major window (the node itself) is used.
        useg = evpool.tile([128, 1, GELEM], F32)
        nc.gpsimd.dma_gather(
            useg[:], zw_d, u_sb[:, 0, :], 128, 128, GELEM,
            single_packet=False, queue_num=0,
        )
        evg = ctx.enter_context(tc.tile_pool(name="evg", bufs=EV_CHUNKS))
        b_tiles = []
        v_gathers = []
        for ch in range(EV_CHUNKS):
            B = evg.tile([128, EV_CC, GELEM], F32, tag="B", name="B")
            v_gathers.append(nc.gpsimd.dma_gather(
                B[:], zw_d, v_sb[:, ch, :], EV_PER_CHUNK, EV_PER_CHUNK, GELEM,
                single_packet=False, queue_num=ch,
            ))
            b_tiles.append(B)

        d2all = evpool.tile([128, C_EV, 1], F32)
        d_ev = evpool.tile([128, C_EV, 1], F32)

        # ---------------- j features  F[p, ct, 0:8] ----------------
        # [1, a, b, c, zx, vx, zy, vy]; padded to 32 for the PE transpose
        F = cpool.tile([128, JT, 32], F32)
        zx = zvj_sb[:, :, 0:1]
        zy = zvj_sb[:, :, 1:2]
        vx = zvj_sb[:, :, 2:3]
        vy = zvj_sb[:, :, 3:4]
        s1 = cpool.tile([128, JT, 1], F32)
        nc.vector.memset(F[:, :, 0:1], 1.0)
        _tt(nc, F[:, :, 1:2], zx, zx, OP.mult)           # a = zx^2 + zy^2
        _tt(nc, s1[:], zy, zy, OP.mult)
        _tt(nc, F[:, :, 1:2], F[:, :, 1:2], s1[:], OP.add)
        s2 = cpool.tile([128, JT, 1], F32)
        _tt(nc, F[:, :, 2:3], zx, vx, OP.mult)           # b = 2(zx vx + zy vy)
        _tt(nc, s2[:], zy, vy, OP.mult)
        _tt(nc, F[:, :, 2:3], F[:, :, 2:3], s2[:], OP.add)
        nc.vector.tensor_scalar_mul(F[:, :, 2:3], F[:, :, 2:3], 2.0)
        s3 = cpool.tile([128, JT, 1], F32)
        _tt(nc, F[:, :, 3:4], vx, vx, OP.mult)           # c = vx^2 + vy^2
        _tt(nc, s3[:], vy, vy, OP.mult)
        _tt(nc, F[:, :, 3:4], F[:, :, 3:4], s3[:], OP.add)
        nc.vector.tensor_copy(F[:, :, 4:5], zx)
        nc.vector.tensor_copy(F[:, :, 5:6], vx)
        nc.vector.tensor_copy(F[:, :, 6:7], zy)
        nc.vector.tensor_copy(F[:, :, 7:8], vy)

        # ---------------- i features  L[p, slot, s, 0:8] ----------------
        # [r, 1, t, t^2, -2x, -2tx, -2y, -2ty]
        L = cpool.tile([128, NSLOT, S, 32], F32)
        izx = zvi_sb[:, :, 0:1]
        izy = zvi_sb[:, :, 1:2]
        ivx = zvi_sb[:, :, 2:3]
        ivy = zvi_sb[:, :, 3:4]
        ia = cpool.tile([128, NSLOT, 1], F32)
        ib = cpool.tile([128, NSLOT, 1], F32)
        ic = cpool.tile([128, NSLOT, 1], F32)
        s4 = cpool.tile([128, NSLOT, 1], F32)
        _tt(nc, ia[:], izx, izx, OP.mult)
        _tt(nc, s4[:], izy, izy, OP.mult)
        _tt(nc, ia[:], ia[:], s4[:], OP.add)
        s5 = cpool.tile([128, NSLOT, 1], F32)
        _tt(nc, ib[:], izx, ivx, OP.mult)
        _tt(nc, s5[:], izy, ivy, OP.mult)
        _tt(nc, ib[:], ib[:], s5[:], OP.add)
        nc.vector.tensor_scalar_mul(ib[:], ib[:], 2.0)
        s6 = cpool.tile([128, NSLOT, 1], F32)
        _tt(nc, ic[:], ivx, ivx, OP.mult)
        _tt(nc, s6[:], ivy, ivy, OP.mult)
        _tt(nc, ic[:], ic[:], s6[:], OP.add)

        def b_i(v):  # [128, NSLOT, 1] -> [128, NSLOT, S, 1]
            return v.unsqueeze(2).to_broadcast([128, NSLOT, S, 1])

        tv = tb.unsqueeze(1).unsqueeze(3).to_broadcast([128, NSLOT, S, 1])
        t2v = t2b.unsqueeze(1).unsqueeze(3).to_broadcast([128, NSLOT, S, 1])

        nc.vector.memset(L[:, :, :, 1:2], 1.0)
        nc.vector.tensor_copy(L[:, :, :, 2:3], tv)
        nc.vector.tensor_copy(L[:, :, :, 3:4], t2v)
        Lx = cpool.tile([128, NSLOT, S, 1], F32)
        _tt(nc, Lx[:], b_i(ivx), tv, OP.mult)            # x_i(s) = zx + vx t
        _tt(nc, Lx[:], Lx[:], b_i(izx), OP.add)
        nc.vector.tensor_scalar_mul(L[:, :, :, 4:5], Lx[:], -2.0)
        _tt(nc, L[:, :, :, 5:6], L[:, :, :, 4:5], tv, OP.mult)
        Ly = cpool.tile([128, NSLOT, S, 1], F32)
        _tt(nc, Ly[:], b_i(ivy), tv, OP.mult)
        _tt(nc, Ly[:], Ly[:], b_i(izy), OP.add)
        nc.vector.tensor_scalar_mul(L[:, :, :, 6:7], Ly[:], -2.0)
        _tt(nc, L[:, :, :, 7:8], L[:, :, :, 6:7], tv, OP.mult)
        Lr = cpool.tile([128, NSLOT, S, 1], F32)
        _tt(nc, L[:, :, :, 0:1], b_i(ib), tv, OP.mult)   # r = a + b t + c t^2
        _tt(nc, L[:, :, :, 0:1], L[:, :, :, 0:1], b_i(ia), OP.add)
        _tt(nc, Lr[:], b_i(ic), t2v, OP.mult)
        _tt(nc, L[:, :, :, 0:1], L[:, :, :, 0:1], Lr[:], OP.add)

        # ---------------- transposes (PE) ----------------
        # transpose copies write float32r directly (rounds for the fp32r
        # matmul; Bacc's generate_event_semaphores legalizes the waits)
        T2 = cpool.tile([8, JCOLS], F32R)                # G_j rows
        L2 = cpool.tile([8, NSLOT * S, 128], F32R)       # F_i(s) rows
        # transposes land 4-up in one PSUM bank so each PSUM->SBUF copy
        # moves [8, 512] (the per-op overhead of 70 tiny copies dominated)
        with tc.tile_pool(name="tp", bufs=4, space="PSUM") as tpp:
            for g0 in range(0, JT, 4):                   # JT % 4 == 0
                pt = tpp.tile([32, 4, 128], F32, tag="pt", name="pt")
                for i in range(4):
                    nc.tensor.transpose(pt[:, i, :], F[:, g0 + i, :], ident[:])
                nc.vector.tensor_copy(
                    T2[:, g0 * 128:(g0 + 4) * 128], pt[0:8, :, :]
                )
            nls = NSLOT * S
            for g0 in range(0, nls, 4):
                ng = min(4, nls - g0)
                pt = tpp.tile([32, 4, 128], F32, tag="pt", name="pt")
                for i in range(ng):
                    slot = g0 + i
                    nc.tensor.transpose(
                        pt[:, i, :], L[:, slot // S, slot % S, :], ident[:]
                    )
                nc.vector.tensor_copy(
                    L2[:, g0:g0 + ng, :], pt[0:8, 0:ng, :]
                )

        # ---------------- main pairwise loop ----------------
        sq_insts = []
        ex_insts = []
        relu_insts = []
        addln2_insts = []
        with tc.tile_pool(name="qp", bufs=4, space="PSUM") as qpool, \
                tc.tile_pool(name="wp", bufs=S) as wpool, \
                tc.tile_pool(name="sp", bufs=2) as spool:
            w_tiles = []
            for s in range(S):
                w = wpool.tile([128, JCOLS], BF16, tag="w", name="w")
                for k0 in range(0, NSLOT, 2):            # relu 2 chunks at once
                    nk = min(2, NSLOT - k0)
                    q = qpool.tile([128, 2, 512], F32, tag="q", name="q")
                    for i in range(nk):
                        k = k0 + i
                        nc.tensor.matmul(
                            q[:, i, :], L2[:, k * S + s, :],
                            T2[:, k * 512:(k + 1) * 512],
                            start=True, stop=True,
                        )
                    relu_insts.append(nc.vector.tensor_scalar_max(
                        w[:, k0 * 512:(k0 + nk) * 512], q[:, 0:nk, :], 0.0
                    ))
                sq_insts.append(nc.scalar.activation(w[:], w[:], AF.Sqrt))
                # diag blocks sit at cols [0:128] (slot 0) and [512:640]
                # (slot 1); adding ln2 to d there makes the single exp pass
                # halve the double-counted within-block pairs (self-pairs
                # contribute 0.5 each, host subtracts)
                for lo, hi in ((0, 128), (512, 640)):
                    addln2_insts.append(nc.vector.tensor_scalar_add(
                        w[:, lo:hi], w[:, lo:hi], LN2
                    ))
                ex_insts.append(nc.scalar.activation(
                    w[:], w[:], AF.Exp, scale=-1.0,
                    accum_out=acc[:, s:s + 1],
                ))
                w_tiles.append(w)

            # ---- event math AFTER the relus in the DVE stream: its inputs
            # (gathers) complete late; without the explicit dep the scheduler
            # hoists these ops early and their sem-waits block the in-order
            # DVE stream for tens of us
            usegb = spool.tile([128, WIN * 4], BF16, tag="ub", name="ub")
            ucast = nc.vector.tensor_copy(usegb[:], useg[:, 0, 0:WIN * 4])
            add_dep_helper(ucast.ins, relu_insts[-1].ins,
                           reason="event DVE after pairwise DVE")
            usegv = (
                usegb.rearrange("p (c w) -> p c w", c=4)
                .unsqueeze(1)
                .to_broadcast([128, EV_CC, 4, WIN])
            )
            ev_sqs = []
            for ch in range(EV_CHUNKS):
                q0 = ch * EV_CC
                B = b_tiles[ch]
                ohv = (
                    oh_sb[:, q0:q0 + EV_CC, :]
                    .unsqueeze(2)
                    .to_broadcast([128, EV_CC, 4, WIN])
                )
                T = spool.tile([128, EV_CC, 4, WIN], BF16, tag="T", name="T")
                _tt(nc, T[:], ohv, usegv, OP.mult)
                zvu = spool.tile([128, EV_CC, 4], F32, tag="zvu", name="zvu")
                nc.vector.tensor_reduce(
                    zvu[:], T[:], axis=mybir.AxisListType.X, op=OP.add
                )

                def uv(c):  # event's u-side component c
                    return zvu[:, :, c:c + 1]

                def bv(c):  # event's v-side component c (col 0 of window c)
                    return B[:, :, c * WIN:c * WIN + 1]

                tse = t_sb[:, q0:q0 + EV_CC].unsqueeze(2)
                shape3 = [128, EV_CC, 1]
                dzx = spool.tile(shape3, F32, tag="dzx", name="dzx")
                dvx = spool.tile(shape3, F32, tag="dvx", name="dvx")
                dzy = spool.tile(shape3, F32, tag="dzy", name="dzy")
                dvy = spool.tile(shape3, F32, tag="dvy", name="dvy")
                _tt(nc, dzx[:], uv(0), bv(0), OP.subtract)
                _tt(nc, dvx[:], uv(2), bv(2), OP.subtract)
                _tt(nc, dvx[:], dvx[:], tse, OP.mult)
                _tt(nc, dzx[:], dzx[:], dvx[:], OP.add)          # dx
                _tt(nc, dzy[:], uv(1), bv(1), OP.subtract)
                _tt(nc, dvy[:], uv(3), bv(3), OP.subtract)
                _tt(nc, dvy[:], dvy[:], tse, OP.mult)
                _tt(nc, dzy[:], dzy[:], dvy[:], OP.add)          # dy
                _tt(nc, dzx[:], dzx[:], dzx[:], OP.mult)
                _tt(nc, dzy[:], dzy[:], dzy[:], OP.mult)
                _tt(nc, d2all[:, q0:q0 + EV_CC, :], dzx[:], dzy[:], OP.add)
                ev_sqs.append(nc.scalar.activation(
                    d_ev[:, q0:q0 + EV_CC, :], d2all[:, q0:q0 + EV_CC, :],
                    AF.Sqrt, accum_out=acc[:, 20 + ch:21 + ch],
                ))

            # Force ACT phase order: all sqrts, then all exps, then the
            # (late-arriving) per-chunk event sqrts
            order = sq_insts + ex_insts + ev_sqs
            for a, b in zip(order[1:], order[:-1]):
                add_dep_helper(a.ins, b.ins, reason="act phase order")

            nc.sync.dma_start(out_p, acc[:])

    nc.compile()
    _CACHE["nc"] = nc
    return nc


def _marshal(inputs):
    z0 = np.asarray(inputs["z0"], dtype=np.float32)
    v0 = np.asarray(inputs["v0"], dtype=np.float32)
    uv = np.asarray(inputs["data_uv"], dtype=np.int32)
    tt = np.asarray(inputs["data_t"], dtype=np.float32)
    t0 = np.float32(np.asarray(inputs["t0"]).reshape(-1)[0])
    tn = np.float32(np.asarray(inputs["tn"]).reshape(-1)[0])

    zv = np.ascontiguousarray(np.concatenate([z0, v0], axis=1)).astype(np.float32)
    dt = np.float32((tn - t0) / np.float32(S))
    tmid = (t0 + (np.arange(S, dtype=np.float32) + np.float32(0.5)) * dt).astype(
        np.float32
    )
    tb = np.ascontiguousarray(np.broadcast_to(tmid, (128, S))).astype(np.float32)
    t2b = (tb * tb).astype(np.float32)

    # windowed table: row n = zv[n:n+16].T (c-major), 256B
    zv_ext = np.vstack([zv, np.zeros((WIN - 1, 4), np.float32)])
    zw = np.zeros((N, GELEM), np.float32)
    for c in range(4):
        for w in range(WIN):
            zw[:, c * WIN + w] = zv_ext[w:w + N, c]

    SENT = np.array([1e4, 1e4, 0.0, 0.0], np.float32)

    E = uv.shape[0]
    assert E == NCORES * EV_PER_CORE
    order = np.argsort(uv[:, 0], kind="stable")
    u_all = uv[order, 0].astype(np.int64)
    v_all = uv[order, 1].astype(np.int64)
    t_all = tt[order]

    def wrap16(x, nops, per_op):
        # [nops*per_op] index list -> [128, nops, per_op//16]: op ch's
        # index k at [k % 16, ch, k // 16], replicated down 8 blocks
        w = x.reshape(nops, per_op // 16, 16).transpose(2, 0, 1)
        return np.ascontiguousarray(np.tile(w, (8, 1, 1)))

    ident_np = np.eye(128, dtype=np.float32)
    in_maps = []
    for k in range(NCORES):
        sl = slice(k * EV_PER_CORE, (k + 1) * EV_PER_CORE)
        us, vs, ts = u_all[sl], v_all[sl], t_all[sl]
        npad = 128 * C_EV - EV_PER_CORE
        upad = np.full(npad, us[-1], np.int64)
        us = np.concatenate([us, upad])
        vs = np.concatenate([vs, upad])          # v = u, t = 0 -> d = 0
        ts = np.concatenate([ts, np.zeros(npad, np.float32)])
        us_m = us.reshape(128, C_EV)
        vs_m = vs.reshape(128, C_EV)
        ts_m = ts.reshape(128, C_EV).astype(np.float32)
        u_start = us_m[:, 0]
        offs = us_m - u_start[:, None]
        assert offs.min() >= 0 and offs.max() < WIN, (
            f"u-window overflow: {offs.max()}"
        )
        oh = (offs[:, :, None] == np.arange(WIN)[None, None, :])
        # v gather chunk ch, list position m = q*128 + p over its 49 cols
        v_list = (
            vs_m.reshape(128, EV_CHUNKS, EV_CC).transpose(1, 2, 0).reshape(-1)
        )
        # pairwise chunk data
        zvi = np.zeros((NSLOT * 128, 4), np.float32)
        zvj = np.zeros((NSLOT * 512, 4), np.float32)
        for sidx, (t, start, ncols, _first) in enumerate(_PLAN[k]):
            zvi[sidx * 128:(sidx + 1) * 128] = zv[t * 128:(t + 1) * 128]
            cj = np.broadcast_to(SENT, (512, 4)).copy()
            cj[:ncols] = zv[start:start + ncols]
            zvj[sidx * 512:(sidx + 1) * 512] = cj
        in_maps.append(
            {
                "zw": zw,
                "zv_j": zvj,
                "zv_i": zvi,
                "ev_u": wrap16(u_start.astype(np.int16), 1, 128),
                "ev_v": wrap16(v_list.astype(np.int16), EV_CHUNKS, EV_PER_CHUNK),
                "ev_oh": _to_bf16(oh.astype(np.float32)),
                "ev_t": ts_m,
                "tb": tb,
                "t2b": t2b,
                "ident": ident_np,
            }
        )
    return in_maps, (float(t0), float(tn), E)


def _to_bf16(x):
    try:
        import ml_dtypes

        return x.astype(ml_dtypes.bfloat16)
    except ImportError:
        # bf16 = upper 16 bits of f32 (round-to-nearest-even)
        xi = x.astype(np.float32).view(np.uint32)
        r = ((xi >> 16) & 1) + 0x7FFF
        return ((xi + r) >> 16).astype(np.uint16)


def _combine(core_outs, beta, t0, tn, E):
    """core_outs: list of [128, 24] float32 partial-sum tensors."""
    exp_sum = 0.0
    ev_sum = 0.0
    for o in core_outs:
        o = np.asarray(o, dtype=np.float64)
        exp_sum += o[:, 0:S].sum()
        ev_sum += o[:, 20:24].sum()
    b = float(beta)
    dt = (tn - t0) / S
    # each core x sample: 2 diag blocks x 128 self-pairs x exp(-ln2) = 128
    exp_sum -= NCORES * S * 128 * 0.5 * 2
    event_intensity = E * b - ev_sum
    non_event = np.exp(b) * exp_sum * dt
    return np.float32(event_intensity - 1.0 * non_event)


def kernel(**inputs) -> np.ndarray:
    from concourse.bass_utils import run_bass_kernel_spmd

    nc = _build()
    in_maps, (t0, tn, E) = _marshal(inputs)
    res = run_bass_kernel_spmd(nc, in_maps, core_ids=list(range(NCORES)))
    beta = float(np.asarray(inputs["beta"]).reshape(-1)[0])
    out = _combine([r["out_p"] for r in res.results], beta, t0, tn, E)
    return np.asarray(out, dtype=np.float32)


# revision 29
# speedup vs baseline: 1.2119x; 1.2119x over previous
"""Trainium2 Bass kernel for BasicEuclideanDistModel (gnn_message_passing).

Math:
  result = sum_e (beta - ||dz_e + dv_e t_e||)
           - dt * sum_{i<j, s} exp(beta - ||z_i(t_s) - z_j(t_s)||)

Device strategy (8 cores, one uniform SPMD program; per-core variation
lives entirely in the input DATA):

* Non-event term, upper-triangular only (~2x less work than full NxN):
  d^2(i,j,s) = F_i(s) . G_j (K=8 fp32r inner product, see below). The
  triangle is cut into 40 column-chunks of 512 (16 "diag" chunks that
  start at a tile's diagonal block + 24 continuations, sentinel-padded
  to 512). Every core gets exactly 2 diag + 3 continuation chunks ->
  [128, 2560] per sample; the host packs per-core i-slot rows (zv_i)
  and j-column node lists (zv_j). Pad columns hold a far-away sentinel
  node so exp(-d) is exactly 0. Diag chunks sit at w[:, 0:256]; their
  exp uses bias=-ln2 (halves the double-counted in-block pairs; the
  128 self-pairs per block contribute 0.5 each, host subtracts).
    F_i(s) = [r_i(s), 1, t_s, t_s^2, -2x_i(s), -2t_s x_i(s), -2y_i(s), -2t_s y_i(s)]
    G_j    = [1,  a_j, b_j, c_j,  zx_j, vx_j, zy_j, vy_j]
  ACT sqrt then exp with fused per-partition row sums (one accum col
  per (s, diag/strip)).

* Event term: events globally sorted by u; each partition owns 196
  consecutive events, so its u values span < 16 consecutive nodes.
  One windowed-row table zw[n] = zv[n:n+16].T (c-major, 256B rows)
  serves both sides:
    - u side: ONE 256B gather per partition (128 descriptors) of the
      window starting at that partition's first u; a [16]-one-hot
      (host input) selects each event's u row on DVE (bf16).
    - v side: one 256B gather per event slot (25088 descriptors, 4
      ops across the 4 SWDGE queues); only the first c-major column
      (the row's own node) is read back.
  DVE distance algebra in f32, ACT sqrt with fused row-sum.

* beta folded in on host: sum exp(beta-d) = e^beta sum exp(-d);
  sum(beta-d) = E beta - sum d. Host combines the 8 cores' [128, 24]
  partial-sum tensors (pure unshard/reduction).
"""

import math
import os

import numpy as np


def _import_concourse():
    try:
        import concourse  # noqa: F401
    except ImportError:
        import sys

        for p in ("/opt/trn_rl_repo", "/root/.axon_site/_ro/trn_rl_repo"):
            if os.path.isdir(p) and p not in sys.path:
                sys.path.insert(0, p)


_import_concourse()

from contextlib import ExitStack  # noqa: E402

import concourse.bacc as bacc  # noqa: E402
import concourse.bass as bass  # noqa: E402
import concourse.mybir as mybir  # noqa: E402
import concourse.tile as tile  # noqa: E402
from concourse.tile_rust import add_dep_helper  # noqa: E402

N = 2048          # nodes
S = 10            # Riemann samples
NCORES = 8
NSLOT = 5         # 512-col j-chunks per core (2 diag + 3 continuation)
JCOLS = NSLOT * 512                  # 2560 j columns per sample
JT = JCOLS // 128                    # 20 column-tiles for G features
EV_PER_CORE = 200000 // NCORES       # 25000 real events per core
C_EV = 196        # event slots per partition (128*196 = 25088 >= 25000)
EV_CHUNKS = 4     # v-side gather ops per core (one per SWDGE queue)
EV_CC = C_EV // EV_CHUNKS            # 49 event columns per chunk
EV_PER_CHUNK = 128 * EV_CC           # 6272
WIN = 4           # u-window nodes (max observed span is 3)
GELEM = 64        # gather element size in f32 (256B rows; first 4*WIN used)
LN2 = math.log(2.0)

F32 = mybir.dt.float32
F32R = mybir.dt.float32r
BF16 = mybir.dt.bfloat16
I16 = mybir.dt.int16
AF = mybir.ActivationFunctionType
OP = mybir.AluOpType

_CACHE: dict = {}


def _tt(nc, out, in0, in1, op):
    return nc.vector.tensor_tensor(out, in0, in1, op=op)


# ---- static chunk assignment (core c -> 5 chunks of the triangle) ----
def _chunk_plan():
    """Returns per-core [(tile, col_start, ncols_real, is_first) x 5].
    Chunk k of tile t covers j-columns [t*128 + k*512, ...) of the strip
    j in [t*128, 2048). Slot 0/1 are diag chunks (start at the tile's
    own block), slots 2-4 are continuations."""
    firsts = []
    conts = []
    for t in range(16):
        w = (16 - t) * 128
        nch = (w + 511) // 512
        for k in range(nch):
            start = t * 128 + k * 512
            ncols = min(512, w - k * 512)
            (firsts if k == 0 else conts).append((t, start, ncols, k == 0))
    assert len(firsts) == 16 and len(conts) == 24
    plan = []
    for c in range(NCORES):
        plan.append([firsts[c], firsts[15 - c]] + conts[3 * c : 3 * c + 3])
    return plan


_PLAN = _chunk_plan()


def _build():
    if "nc" in _CACHE:
        return _CACHE["nc"]

    nc = bacc.Bacc(
        "TRN2", target_bir_lowering=False, debug=False, enable_asserts=False,
        num_swdge_queues=4,
    )

    zw_d = nc.dram_tensor("zw", [N, GELEM], F32, kind="ExternalInput").ap()
    zvj_d = nc.dram_tensor("zv_j", [JCOLS, 4], F32, kind="ExternalInput").ap()
    zvi_d = nc.dram_tensor("zv_i", [NSLOT * 128, 4], F32, kind="ExternalInput").ap()
    # int16 indices, dma_gather wrap: op ch's index k lives at
    # [k % 16, ch, k // 16], replicated down all 8 blocks of 16 partitions
    ev_u = nc.dram_tensor("ev_u", [128, 1, 128 // 16], I16, kind="ExternalInput").ap()
    ev_v = nc.dram_tensor(
        "ev_v", [128, EV_CHUNKS, EV_PER_CHUNK // 16], I16, kind="ExternalInput"
    ).ap()
    ev_oh = nc.dram_tensor("ev_oh", [128, C_EV, WIN], BF16, kind="ExternalInput").ap()
    ev_t = nc.dram_tensor("ev_t", [128, C_EV], F32, kind="ExternalInput").ap()
    tb_d = nc.dram_tensor("tb", [128, S], F32, kind="ExternalInput").ap()
    t2b_d = nc.dram_tensor("t2b", [128, S], F32, kind="ExternalInput").ap()
    ident_d = nc.dram_tensor("ident", [128, 128], F32, kind="ExternalInput").ap()
    out_p = nc.dram_tensor("out_p", [128, 24], F32, kind="ExternalOutput").ap()

    with tile.TileContext(nc) as tc, ExitStack() as ctx:
        cpool = ctx.enter_context(tc.tile_pool(name="const", bufs=1))
        evpool = ctx.enter_context(tc.tile_pool(name="ev", bufs=1))

        # ---------------- input loads ----------------
        # pairwise-critical loads go FIRST on the Sync (HWDGE) queue so their
        # completion sems are never sequenced after the (slow) gather sems;
        # the event index loads ride the Pool engine itself (SWDGE), putting
        # them in-stream with the gathers that consume them
        zvj_sb = cpool.tile([128, JT, 4], F32)       # this core's j columns
        nc.sync.dma_start(zvj_sb[:], zvj_d.rearrange("(c p) d -> p c d", p=128))
        zvi_sb = cpool.tile([128, NSLOT, 4], F32)    # this core's i slots
        nc.sync.dma_start(zvi_sb[:], zvi_d.rearrange("(c p) d -> p c d", p=128))
        tb = cpool.tile([128, S], F32)
        nc.sync.dma_start(tb[:], tb_d)
        t2b = cpool.tile([128, S], F32)
        nc.sync.dma_start(t2b[:], t2b_d)
        # identity comes from the host: building it with make_identity would
        # occupy the gpsimd engine ahead of the gather descriptor generation
        ident = cpool.tile([128, 128], F32)
        nc.sync.dma_start(ident[:], ident_d)
        u_sb = evpool.tile([128, 1, 128 // 16], I16)
        nc.gpsimd.dma_start(u_sb[:], ev_u)
        v_sb = evpool.tile([128, EV_CHUNKS, EV_PER_CHUNK // 16], I16)
        nc.gpsimd.dma_start(v_sb[:], ev_v)
        oh_sb = evpool.tile([128, C_EV, WIN], BF16)
        nc.sync.dma_start(oh_sb[:], ev_oh)
        t_sb = evpool.tile([128, C_EV], F32)
        nc.sync.dma_start(t_sb[:], ev_t)

        acc = cpool.tile([128, 24], F32)
        nc.vector.memset(acc[:], 0.0)

        # ---------------- event gathers (all Pool-engine work upfront) ----
        # u side: one 256B window row per partition (nodes [u0, u0+16),
        # c-major [4, 16]); v side: one row per event slot, of which only
        # column 0 of the c-major window (the node itself) is used.
        useg = evpool.tile([128, 1, GELEM], F32)
        nc.gpsimd.dma_gather(
            useg[:], zw_d, u_sb[:, 0, :], 128, 128, GELEM,
            single_packet=False, queue_num=0,
        )
        evg = ctx.enter_context(tc.tile_pool(name="evg", bufs=EV_CHUNKS))
        b_tiles = []
        v_gathers = []
        for ch in range(EV_CHUNKS):
            B = evg.tile([128, EV_CC, GELEM], F32, tag="B", name="B")
            v_gathers.append(nc.gpsimd.dma_gather(
                B[:], zw_d, v_sb[:, ch, :], EV_PER_CHUNK, EV_PER_CHUNK, GELEM,
                single_packet=False, queue_num=ch,
            ))
            b_tiles.append(B)

        d2all = evpool.tile([128, C_EV, 1], F32)
        d_ev = evpool.tile([128, C_EV, 1], F32)

        # ---------------- j features  F[p, ct, 0:8] ----------------
        # [1, a, b, c, zx, vx, zy, vy]; padded to 32 for the PE transpose
        F = cpool.tile([128, JT, 32], F32)
        zx = zvj_sb[:, :, 0:1]
        zy = zvj_sb[:, :, 1:2]
        vx = zvj_sb[:, :, 2:3]
        vy = zvj_sb[:, :, 3:4]
        s1 = cpool.tile([128, JT, 1], F32)
        nc.vector.memset(F[:, :, 0:1], 1.0)
        _tt(nc, F[:, :, 1:2], zx, zx, OP.mult)           # a = zx^2 + zy^2
        _tt(nc, s1[:], zy, zy, OP.mult)
        _tt(nc, F[:, :, 1:2], F[:, :, 1:2], s1[:], OP.add)
        s2 = cpool.tile([128, JT, 1], F32)
        _tt(nc, F[:, :, 2:3], zx, vx, OP.mult)           # b = 2(zx vx + zy vy)
        _tt(nc, s2[:], zy, vy, OP.mult)
        _tt(nc, F[:, :, 2:3], F[:, :, 2:3], s2[:], OP.add)
        nc.vector.tensor_scalar_mul(F[:, :, 2:3], F[:, :, 2:3], 2.0)
        s3 = cpool.tile([128, JT, 1], F32)
        _tt(nc, F[:, :, 3:4], vx, vx, OP.mult)           # c = vx^2 + vy^2
        _tt(nc, s3[:], vy, vy, OP.mult)
        _tt(nc, F[:, :, 3:4], F[:, :, 3:4], s3[:], OP.add)
        nc.vector.tensor_copy(F[:, :, 4:5], zx)
        nc.vector.tensor_copy(F[:, :, 5:6], vx)
        nc.vector.tensor_copy(F[:, :, 6:7], zy)
        nc.vector.tensor_copy(F[:, :, 7:8], vy)

        # ---------------- i features  L[p, slot, s, 0:8] ----------------
        # [r, 1, t, t^2, -2x, -2tx, -2y, -2ty]
        L = cpool.tile([128, NSLOT, S, 32], F32)
        izx = zvi_sb[:, :, 0:1]
        izy = zvi_sb[:, :, 1:2]
        ivx = zvi_sb[:, :, 2:3]
        ivy = zvi_sb[:, :, 3:4]
        ia = cpool.tile([128, NSLOT, 1], F32)
        ib = cpool.tile([128, NSLOT, 1], F32)
        ic = cpool.tile([128, NSLOT, 1], F32)
        s4 = cpool.tile([128, NSLOT, 1], F32)
        _tt(nc, ia[:], izx, izx, OP.mult)
        _tt(nc, s4[:], izy, izy, OP.mult)
        _tt(nc, ia[:], ia[:], s4[:], OP.add)
        s5 = cpool.tile([128, NSLOT, 1], F32)
        _tt(nc, ib[:], izx, ivx, OP.mult)
        _tt(nc, s5[:], izy, ivy, OP.mult)
        _tt(nc, ib[:], ib[:], s5[:], OP.add)
        nc.vector.tensor_scalar_mul(ib[:], ib[:], 2.0)
        s6 = cpool.tile([128, NSLOT, 1], F32)
        _tt(nc, ic[:], ivx, ivx, OP.mult)
        _tt(nc, s6[:], ivy, ivy, OP.mult)
        _tt(nc, ic[:], ic[:], s6[:], OP.add)

        def b_i(v):  # [128, NSLOT, 1] -> [128, NSLOT, S, 1]
            return v.unsqueeze(2).to_broadcast([128, NSLOT, S, 1])

        tv = tb.unsqueeze(1).unsqueeze(3).to_broadcast([128, NSLOT, S, 1])
        t2v = t2b.unsqueeze(1).unsqueeze(3).to_broadcast([128, NSLOT, S, 1])

        nc.vector.memset(L[:, :, :, 1:2], 1.0)
        nc.vector.tensor_copy(L[:, :, :, 2:3], tv)
        nc.vector.tensor_copy(L[:, :, :, 3:4], t2v)
        Lx = cpool.tile([128, NSLOT, S, 1], F32)
        _tt(nc, Lx[:], b_i(ivx), tv, OP.mult)            # x_i(s) = zx + vx t
        _tt(nc, Lx[:], Lx[:], b_i(izx), OP.add)
        nc.vector.tensor_scalar_mul(L[:, :, :, 4:5], Lx[:], -2.0)
        _tt(nc, L[:, :, :, 5:6], L[:, :, :, 4:5], tv, OP.mult)
        Ly = cpool.tile([128, NSLOT, S, 1], F32)
        _tt(nc, Ly[:], b_i(ivy), tv, OP.mult)
        _tt(nc, Ly[:], Ly[:], b_i(izy), OP.add)
        nc.vector.tensor_scalar_mul(L[:, :, :, 6:7], Ly[:], -2.0)
        _tt(nc, L[:, :, :, 7:8], L[:, :, :, 6:7], tv, OP.mult)
        Lr = cpool.tile([128, NSLOT, S, 1], F32)
        _tt(nc, L[:, :, :, 0:1], b_i(ib), tv, OP.mult)   # r = a + b t + c t^2
        _tt(nc, L[:, :, :, 0:1], L[:, :, :, 0:1], b_i(ia), OP.add)
        _tt(nc, Lr[:], b_i(ic), t2v, OP.mult)
        _tt(nc, L[:, :, :, 0:1], L[:, :, :, 0:1], Lr[:], OP.add)

        # ---------------- transposes (PE) ----------------
        # transpose copies write float32r directly (rounds for the fp32r
        # matmul; Bacc's generate_event_semaphores legalizes the waits)
        T2 = cpool.tile([8, JCOLS], F32R)                # G_j rows
        L2 = cpool.tile([8, NSLOT * S, 128], F32R)       # F_i(s) rows
        # transposes land 4-up in one PSUM bank so each PSUM->SBUF copy
        # moves [8, 512] (the per-op overhead of 70 tiny copies dominated)
        with tc.tile_pool(name="tp", bufs=4, space="PSUM") as tpp:
            for g0 in range(0, JT, 4):                   # JT % 4 == 0
                pt = tpp.tile([32, 4, 128], F32, tag="pt", name="pt")
                for i in range(4):
                    nc.tensor.transpose(pt[:, i, :], F[:, g0 + i, :], ident[:])
                nc.vector.tensor_copy(
                    T2[:, g0 * 128:(g0 + 4) * 128], pt[0:8, :, :]
                )
            nls = NSLOT * S
            for g0 in range(0, nls, 4):
                ng = min(4, nls - g0)
                pt = tpp.tile([32, 4, 128], F32, tag="pt", name="pt")
                for i in range(ng):
                    slot = g0 + i
                    nc.tensor.transpose(
                        pt[:, i, :], L[:, slot // S, slot % S, :], ident[:]
                    )
                nc.vector.tensor_copy(
                    L2[:, g0:g0 + ng, :], pt[0:8, 0:ng, :]
                )

        # ---------------- main pairwise loop ----------------
        sq_insts = []
        ex_insts = []
        relu_insts = []
        addln2_insts = []
        with tc.tile_pool(name="qp", bufs=4, space="PSUM") as qpool, \
                tc.tile_pool(name="wp", bufs=S) as wpool, \
                tc.tile_pool(name="sp", bufs=2) as spool:
            w_tiles = []

            def _emit_diag_exp(sp):
                for lo, hi in ((0, 128), (512, 640)):
                    addln2_insts.append(nc.vector.tensor_scalar_add(
                        w_tiles[sp][:, lo:hi], w_tiles[sp][:, lo:hi], LN2
                    ))
                ex_insts.append(nc.scalar.activation(
                    w_tiles[sp][:], w_tiles[sp][:], AF.Exp, scale=-1.0,
                    accum_out=acc[:, sp:sp + 1],
                ))

            for s in range(S):
                w = wpool.tile([128, JCOLS], BF16, tag="w", name="w")
                for k0 in range(0, NSLOT, 2):            # relu 2 chunks at once
                    nk = min(2, NSLOT - k0)
                    q = qpool.tile([128, 2, 512], F32, tag="q", name="q")
                    for i in range(nk):
                        k = k0 + i
                        nc.tensor.matmul(
                            q[:, i, :], L2[:, k * S + s, :],
                            T2[:, k * 512:(k + 1) * 512],
                            start=True, stop=True,
                        )
                    relu_insts.append(nc.vector.tensor_scalar_max(
                        w[:, k0 * 512:(k0 + nk) * 512], q[:, 0:nk, :], 0.0
                    ))
                sq_insts.append(nc.scalar.activation(w[:], w[:], AF.Sqrt))
                w_tiles.append(w)
                # diag blocks sit at cols [0:128] (slot 0) and [512:640]
                # (slot 1); adding ln2 to d there makes the single exp pass
                # halve the double-counted within-block pairs (self-pairs
                # contribute 0.5 each, host subtracts). The adds for sample
                # s-1 are emitted AFTER relus_s so the DVE never idles
                # waiting on sqrt_s; each exp is emitted after its adds.
                if s > 0:
                    _emit_diag_exp(s - 1)
            _emit_diag_exp(S - 1)

            # ---- event math AFTER the relus in the DVE stream: its inputs
            # (gathers) complete late; without the explicit deps the
            # scheduler (whose cost model thinks descgen is ~7x faster than
            # reality) hoists event waits early and blocks the in-order DVE
            # stream for tens of us. The dummy memsets are wait SINKS: each
            # has a free wait slot, so surplus cross-engine waits from the
            # event ops legalize onto them instead of onto main-loop ops.
            sink = spool.tile([128, 8], F32, tag="sink", name="sink")
            prev = addln2_insts[-1]
            sink_insts = []
            for i in range(8):
                si = nc.vector.memset(sink[:, i:i + 1], 0.0)
                add_dep_helper(si.ins, prev.ins, reason="wait sink chain")
                sink_insts.append(si)
                prev = si
            usegb = spool.tile([128, WIN * 4], BF16, tag="ub", name="ub")
            ucast = nc.vector.tensor_copy(usegb[:], useg[:, 0, 0:WIN * 4])
            add_dep_helper(ucast.ins, prev.ins,
                           reason="event DVE after pairwise DVE")
            usegv = (
                usegb.rearrange("p (c w) -> p c w", c=4)
                .unsqueeze(1)
                .to_broadcast([128, EV_CC, 4, WIN])
            )
            ev_sqs = []
            for ch in range(EV_CHUNKS):
                q0 = ch * EV_CC
                B = b_tiles[ch]
                ohv = (
                    oh_sb[:, q0:q0 + EV_CC, :]
                    .unsqueeze(2)
                    .to_broadcast([128, EV_CC, 4, WIN])
                )
                T = spool.tile([128, EV_CC, 4, WIN], BF16, tag="T", name="T")
                _tt(nc, T[:], ohv, usegv, OP.mult)
                zvu = spool.tile([128, EV_CC, 4], F32, tag="zvu", name="zvu")
                nc.vector.tensor_reduce(
                    zvu[:], T[:], axis=mybir.AxisListType.X, op=OP.add
                )

                def uv(c):  # event's u-side component c
                    return zvu[:, :, c:c + 1]

                def bv(c):  # event's v-side component c (col 0 of window c)
                    return B[:, :, c * WIN:c * WIN + 1]

                tse = t_sb[:, q0:q0 + EV_CC].unsqueeze(2)
                shape3 = [128, EV_CC, 1]
                dzx = spool.tile(shape3, F32, tag="dzx", name="dzx")
                dvx = spool.tile(shape3, F32, tag="dvx", name="dvx")
                dzy = spool.tile(shape3, F32, tag="dzy", name="dzy")
                dvy = spool.tile(shape3, F32, tag="dvy", name="dvy")
                _tt(nc, dzx[:], uv(0), bv(0), OP.subtract)
                _tt(nc, dvx[:], uv(2), bv(2), OP.subtract)
                _tt(nc, dvx[:], dvx[:], tse, OP.mult)
                _tt(nc, dzx[:], dzx[:], dvx[:], OP.add)          # dx
                _tt(nc, dzy[:], uv(1), bv(1), OP.subtract)
                _tt(nc, dvy[:], uv(3), bv(3), OP.subtract)
                _tt(nc, dvy[:], dvy[:], tse, OP.mult)
                _tt(nc, dzy[:], dzy[:], dvy[:], OP.add)          # dy
                _tt(nc, dzx[:], dzx[:], dzx[:], OP.mult)
                _tt(nc, dzy[:], dzy[:], dzy[:], OP.mult)
                _tt(nc, d2all[:, q0:q0 + EV_CC, :], dzx[:], dzy[:], OP.add)
                ev_sqs.append(nc.scalar.activation(
                    d_ev[:, q0:q0 + EV_CC, :], d2all[:, q0:q0 + EV_CC, :],
                    AF.Sqrt, accum_out=acc[:, 20 + ch:21 + ch],
                ))

            # Force ACT phase order: all sqrts, then all exps, then the
            # (late-arriving) per-chunk event sqrts
            order = sq_insts + ex_insts + ev_sqs
            for a, b in zip(order[1:], order[:-1]):
                add_dep_helper(a.ins, b.ins, reason="act phase order")

            nc.sync.dma_start(out_p, acc[:])

    nc.compile()
    _CACHE["nc"] = nc
    return nc


def _marshal(inputs):
    z0 = np.asarray(inputs["z0"], dtype=np.float32)
    v0 = np.asarray(inputs["v0"], dtype=np.float32)
    uv = np.asarray(inputs["data_uv"], dtype=np.int32)
    tt = np.asarray(inputs["data_t"], dtype=np.float32)
    t0 = np.float32(np.asarray(inputs["t0"]).reshape(-1)[0])
    tn = np.float32(np.asarray(inputs["tn"]).reshape(-1)[0])

    zv = np.ascontiguousarray(np.concatenate([z0, v0], axis=1)).astype(np.float32)
    dt = np.float32((tn - t0) / np.float32(S))
    tmid = (t0 + (np.arange(S, dtype=np.float32) + np.float32(0.5)) * dt).astype(
        np.float32
    )
    tb = np.ascontiguousarray(np.broadcast_to(tmid, (128, S))).astype(np.float32)
    t2b = (tb * tb).astype(np.float32)

    # windowed table: row n = zv[n:n+16].T (c-major), 256B
    zv_ext = np.vstack([zv, np.zeros((WIN - 1, 4), np.float32)])
    zw = np.zeros((N, GELEM), np.float32)
    for c in range(4):
        for w in range(WIN):
            zw[:, c * WIN + w] = zv_ext[w:w + N, c]

    SENT = np.array([1e4, 1e4, 0.0, 0.0], np.float32)

    E = uv.shape[0]
    assert E == NCORES * EV_PER_CORE
    order = np.argsort(uv[:, 0], kind="stable")
    u_all = uv[order, 0].astype(np.int64)
    v_all = uv[order, 1].astype(np.int64)
    t_all = tt[order]

    def wrap16(x, nops, per_op):
        # [nops*per_op] index list -> [128, nops, per_op//16]: op ch's
        # index k at [k % 16, ch, k // 16], replicated down 8 blocks
        w = x.reshape(nops, per_op // 16, 16).transpose(2, 0, 1)
        return np.ascontiguousarray(np.tile(w, (8, 1, 1)))

    ident_np = np.eye(128, dtype=np.float32)
    in_maps = []
    for k in range(NCORES):
        sl = slice(k * EV_PER_CORE, (k + 1) * EV_PER_CORE)
        us, vs, ts = u_all[sl], v_all[sl], t_all[sl]
        npad = 128 * C_EV - EV_PER_CORE
        upad = np.full(npad, us[-1], np.int64)
        us = np.concatenate([us, upad])
        vs = np.concatenate([vs, upad])          # v = u, t = 0 -> d = 0
        ts = np.concatenate([ts, np.zeros(npad, np.float32)])
        us_m = us.reshape(128, C_EV)
        vs_m = vs.reshape(128, C_EV)
        ts_m = ts.reshape(128, C_EV).astype(np.float32)
        u_start = us_m[:, 0]
        offs = us_m - u_start[:, None]
        assert offs.min() >= 0 and offs.max() < WIN, (
            f"u-window overflow: {offs.max()}"
        )
        oh = (offs[:, :, None] == np.arange(WIN)[None, None, :])
        # v gather chunk ch, list position m = q*128 + p over its 49 cols
        v_list = (
            vs_m.reshape(128, EV_CHUNKS, EV_CC).transpose(1, 2, 0).reshape(-1)
        )
        # pairwise chunk data
        zvi = np.zeros((NSLOT * 128, 4), np.float32)
        zvj = np.zeros((NSLOT * 512, 4), np.float32)
        for sidx, (t, start, ncols, _first) in enumerate(_PLAN[k]):
            zvi[sidx * 128:(sidx + 1) * 128] = zv[t * 128:(t + 1) * 128]
            cj = np.broadcast_to(SENT, (512, 4)).copy()
            cj[:ncols] = zv[start:start + ncols]
            zvj[sidx * 512:(sidx + 1) * 512] = cj
        in_maps.append(
            {
                "zw": zw,
                "zv_j": zvj,
                "zv_i": zvi,
                "ev_u": wrap16(u_start.astype(np.int16), 1, 128),
                "ev_v": wrap16(v_list.astype(np.int16), EV_CHUNKS, EV_PER_CHUNK),
                "ev_oh": _to_bf16(oh.astype(np.float32)),
                "ev_t": ts_m,
                "tb": tb,
                "t2b": t2b,
                "ident": ident_np,
            }
        )
    return in_maps, (float(t0), float(tn), E)


def _to_bf16(x):
    try:
        import ml_dtypes

        return x.astype(ml_dtypes.bfloat16)
    except ImportError:
        # bf16 = upper 16 bits of f32 (round-to-nearest-even)
        xi = x.astype(np.float32).view(np.uint32)
        r = ((xi >> 16) & 1) + 0x7FFF
        return ((xi + r) >> 16).astype(np.uint16)


def _combine(core_outs, beta, t0, tn, E):
    """core_outs: list of [128, 24] float32 partial-sum tensors."""
    exp_sum = 0.0
    ev_sum = 0.0
    for o in core_outs:
        o = np.asarray(o, dtype=np.float64)
        exp_sum += o[:, 0:S].sum()
        ev_sum += o[:, 20:24].sum()
    b = float(beta)
    dt = (tn - t0) / S
    # each core x sample: 2 diag blocks x 128 self-pairs x exp(-ln2) = 128
    exp_sum -= NCORES * S * 128 * 0.5 * 2
    event_intensity = E * b - ev_sum
    non_event = np.exp(b) * exp_sum * dt
    return np.float32(event_intensity - 1.0 * non_event)


def kernel(**inputs) -> np.ndarray:
    from concourse.bass_utils import run_bass_kernel_spmd

    nc = _build()
    in_maps, (t0, tn, E) = _marshal(inputs)
    res = run_bass_kernel_spmd(nc, in_maps, core_ids=list(range(NCORES)))
    beta = float(np.asarray(inputs["beta"]).reshape(-1)[0])
    out = _combine([r["out_p"] for r in res.results], beta, t0, tn, E)
    return np.asarray(out, dtype=np.float32)
